# revision 19
# baseline (speedup 1.0000x reference)
"""Causal self-attention (B=4, S=2048, D=1024, H=16, hd=64) on 8 TRN2 cores.

Sharding: core c = (batch b = c//2, head-group g = c%2); each core computes
8 heads for one batch. Out-projection partials are summed on host (the only
cross-shard reduction).

Device kernel layout (all matmul contractions have the contracted dim on
SBUF partitions; everything stays transposed so no on-device transposes):
  qT,kT  [64*2heads, S]  = wqkvT-chunk.T @ xT          (stationary weights)
  v_aug  [S-block, 8*65] = xT-chunk.T @ wvT (+ ones col per head for sums)
  sT     [j 128, i 512]  = kT-slice.T @ qT-slice        (2 heads row-packed)
  pT     = exp(sT/8)  bf16 via ACT; diagonal blocks masked by DVE mask-mul
  outT   [65, i]        += v_aug.T @ pT   (row 64 accumulates softmax sums)
  attnT  = outT * bcast(1/sums)           (gpsimd partition_broadcast)
  out    [s 128, e]      = attnT-chunk.T @ woutT-chunk  (accum over c-chunks)

Scheduling: the exp on the scalar engine is the rate limiter of the
attention inner loop (~1.07us per 128-k-row block vs ~0.75us of PE work),
so all non-attention PE work (QKV projection groups, V blocks, out-proj
groups) is emitted as FILLER UNITS spread inside the attention jb-loops;
the PE then never idles and its clock stays at the full p-state. Input
DMAs are issued in dependency-priority order (pair-0 qk weights + first
half of xT first) so the first exp lands ~20us into the kernel. All
PSUM->SBUF copies run on DVE, keeping the scalar engine exp-only.
"""
import sys
import os

sys.path.insert(0, "/opt/trn_rl_repo")

import numpy as np
import ml_dtypes
from contextlib import ExitStack

S = 2048
D = 1024
HL = 8          # heads per core
HD = 64
PAIRS = 4       # head pairs per core
NIB = 4         # i-blocks of 512
N_CORES = 8

_CACHE = {}
LAST_EXEC_TIME_NS = None


def _build():
    import concourse.tile as tile
    import concourse.mybir as mybir
    from concourse import bacc

    bf = mybir.dt.bfloat16
    f32 = mybir.dt.float32
    EXP = mybir.ActivationFunctionType.Exp
    GE = mybir.AluOpType.is_ge

    nc = bacc.Bacc("TRN2", target_bir_lowering=False, debug=False,
                   num_devices=N_CORES)
    xT_d = nc.dram_tensor("xT", [D, S], bf, kind="ExternalInput").ap()
    # host-side column layout: [q0|k0 (256) | v (512) | q1 k1 q2 k2 q3 k3]
    wqkvT_d = nc.dram_tensor("wqkvT", [D, 3 * 512], bf,
                             kind="ExternalInput").ap()
    woutT_d = nc.dram_tensor("woutT", [512, D], bf, kind="ExternalInput").ap()
    out_d = nc.dram_tensor("out", [S, D], f32, kind="ExternalOutput").ap()

    xv = xT_d.rearrange("(c p) s -> p c s", p=128)          # [128, 8, 2048]
    wv_ = wqkvT_d.rearrange("(c p) n -> p c n", p=128)      # [128, 8, 1536]
    wo_ = woutT_d.rearrange("(c p) e -> p c e", p=128)      # [128, 4, 1024]

    with tile.TileContext(nc) as tc, ExitStack() as ctx:
        sb = ctx.enter_context(tc.tile_pool(name="sb", bufs=1))
        # PSUM budget (8 banks): mm 2x[128,1024] (4) + acc 2x[65,512] (2)
        # + unit 2x[128,512] (2).
        mm = ctx.enter_context(tc.tile_pool(name="mm", bufs=2, space="PSUM"))
        acc = ctx.enter_context(tc.tile_pool(name="acc", bufs=2,
                                             space="PSUM"))
        unit = ctx.enter_context(tc.tile_pool(name="unit", bufs=2,
                                              space="PSUM"))
        pp = ctx.enter_context(tc.tile_pool(name="pp", bufs=8))
        rsp = ctx.enter_context(tc.tile_pool(name="rsp", bufs=4))
        bcsp = ctx.enter_context(tc.tile_pool(name="bcsp", bufs=4))
        osbp = ctx.enter_context(tc.tile_pool(name="osbp", bufs=4))

        # ---- persistent SBUF tiles -------------------------------------
        xt0 = sb.tile([128, 8, 512], bf, tag="xt0", name="xt0")
        xt1 = sb.tile([128, 8, 512], bf, tag="xt1", name="xt1")
        xt23 = sb.tile([128, 8, 1024], bf, tag="xt23", name="xt23")
        wqk0 = sb.tile([128, 8, 2, 128], bf, tag="wqk0", name="wqk0")
        wvt = sb.tile([128, 8, 512], bf, tag="wvt", name="wvt")
        wqkR = sb.tile([128, 8, 6, 128], bf, tag="wqkR", name="wqkR")
        wout = sb.tile([128, 4, 1024], bf, tag="wout", name="wout")
        qT = [sb.tile([128, S], bf, tag=f"qT{p}", name=f"qT{p}")
              for p in range(PAIRS)]
        kT = [sb.tile([128, S], bf, tag=f"kT{p}", name=f"kT{p}")
              for p in range(PAIRS)]
        vaug = [sb.tile([128, HL, HD + 1], bf, tag=f"vaug{s}",
                        name=f"vaug{s}") for s in range(16)]
        attnT = [sb.tile([128, S], bf, tag=f"attnT{p}", name=f"attnT{p}")
                 for p in range(PAIRS)]
        masks = [sb.tile([128, 512], bf, tag=f"mask{m}", name=f"mask{m}")
                 for m in range(4)]

        scratch = sb.tile([128, 512], bf, tag="warm", name="warm")

        # ---- init (gpsimd; independent of DMAs). Order: warm-up scratch
        # first (unblocks PE clock warm-up), masks next (needed by the
        # first diagonal-block mask-mul ~17us in), vaug last.
        nc.gpsimd.memset(scratch[:], 0.0)
        for m in range(4):
            nc.gpsimd.memset(masks[m][:], 1.0)
            nc.gpsimd.affine_select(
                out=masks[m][:], in_=masks[m][:], compare_op=GE, fill=0.0,
                base=-128 * m, channel_multiplier=-1, pattern=[[1, 512]])
        for s in range(16):
            nc.gpsimd.memset(vaug[s][:], 1.0)

        # ---- input DMAs in dependency-priority order -------------------
        nc.sync.dma_start(
            wqk0[:], wv_[:, :, 0:256].rearrange("p c (w n) -> p c w n",
                                                n=128))
        nc.sync.dma_start(xt0[:], xv[:, :, 0:512])
        nc.sync.dma_start(wvt[:], wv_[:, :, 256:768])
        nc.sync.dma_start(xt1[:], xv[:, :, 512:1024])
        nc.sync.dma_start(xt23[:], xv[:, :, 1024:2048])
        nc.sync.dma_start(
            wqkR[:], wv_[:, :, 768:1536].rearrange("p c (w n) -> p c w n",
                                                   n=128))
        nc.sync.dma_start(wout[:], wo_[:])

        def xt_rhs(dc, sc):
            if sc == 0:
                return xt0[:, dc, :]
            if sc == 1:
                return xt1[:, dc, :]
            return xt23[:, dc, 512 * (sc % 2):512 * (sc % 2 + 1)]

        def xt_vlhs(dc, sblk):
            sc = sblk // 4
            r = 128 * (sblk % 4)
            if sc == 0:
                return xt0[:, dc, r:r + 128]
            if sc == 1:
                return xt1[:, dc, r:r + 128]
            base = 512 * (sc % 2) + r
            return xt23[:, dc, base:base + 128]

        def wqk_lhs(pair, w, dc):
            if pair == 0:
                return wqk0[:, dc, w, :]
            return wqkR[:, dc, 2 * (pair - 1) + w, :]

        # ---- filler units (each = one PSUM group + a DVE drain) --------
        def v_unit(sblk):
            def emit():
                ps = unit.tile([128, 512], f32, tag="u", name=f"vps{sblk}")
                for dc in range(8):
                    nc.tensor.matmul(ps[:], lhsT=xt_vlhs(dc, sblk),
                                     rhs=wvt[:, dc, :],
                                     start=(dc == 0), stop=(dc == 7))
                nc.vector.tensor_copy(
                    vaug[sblk][:, :, 0:64],
                    ps[:].rearrange("p (h d) -> p h d", h=HL))
            return emit

        def qk_unit(pair, w, sc):
            def emit():
                dest = qT[pair] if w == 0 else kT[pair]
                ps = unit.tile([128, 512], f32, tag="u",
                               name=f"qkps{pair}{w}{sc}")
                for dc in range(8):
                    nc.tensor.matmul(ps[:], lhsT=wqk_lhs(pair, w, dc),
                                     rhs=xt_rhs(dc, sc),
                                     start=(dc == 0), stop=(dc == 7))
                nc.vector.tensor_copy(dest[:, 512 * sc:512 * (sc + 1)],
                                      ps[:])
            return emit

        osb_tiles = {}

        def op_unit(sblk, eh, alt_pool=False, split_dma=False):
            def emit():
                if eh == 0:
                    osb_tiles[sblk] = osbp.tile([128, D], f32, tag="osb",
                                                name=f"osb{sblk}")
                osb = osb_tiles[sblk]
                if alt_pool:
                    pst = mm.tile([128, 1024], f32, tag="mm",
                                  name=f"ops{sblk}{eh}")
                    ps_ap = pst[:, 0:512]
                else:
                    pst = unit.tile([128, 512], f32, tag="u",
                                    name=f"ops{sblk}{eh}")
                    ps_ap = pst[:]
                for cc in range(4):
                    nc.tensor.matmul(
                        ps_ap,
                        lhsT=attnT[cc][:, 128 * sblk:128 * (sblk + 1)],
                        rhs=wout[:, cc, 512 * eh:512 * (eh + 1)],
                        start=(cc == 0), stop=(cc == 3))
                nc.vector.tensor_copy(osb[:, 512 * eh:512 * (eh + 1)],
                                      ps_ap)
                if split_dma:
                    nc.sync.dma_start(
                        out_d[128 * sblk:128 * (sblk + 1),
                              512 * eh:512 * (eh + 1)],
                        osb[:, 512 * eh:512 * (eh + 1)])
                elif eh == 1:
                    nc.sync.dma_start(
                        out_d[128 * sblk:128 * (sblk + 1), :], osb[:])
            return emit

        # ---- attention -------------------------------------------------
        def emit_qkexp(pair, ib, jb):
            off = max(0, 128 * (jb - 4 * ib))
            s2 = mm.tile([128, 1024], f32, tag="mm",
                         name=f"s2_{pair}{ib}{jb}")
            for h01 in range(2):
                r0, r1 = 64 * h01, 64 * (h01 + 1)
                nc.tensor.matmul(
                    s2[:, 512 * h01 + off:512 * (h01 + 1)],
                    lhsT=kT[pair][r0:r1, 128 * jb:128 * (jb + 1)],
                    rhs=qT[pair][r0:r1, 512 * ib + off:512 * (ib + 1)],
                    start=True, stop=True)
            pX = pp.tile([128, 1024], bf, tag="pp", name=f"pX{pair}{ib}{jb}")
            s3 = s2[:].rearrange("p (h i) -> p h i", h=2)
            p3 = pX[:].rearrange("p (h i) -> p h i", h=2)
            nc.scalar.activation(p3[:, :, off:512], s3[:, :, off:512],
                                 EXP, scale=0.125)
            if jb >= 4 * ib:
                m = jb - 4 * ib
                nc.vector.tensor_mul(
                    p3[:, :, off:512], p3[:, :, off:512],
                    masks[m][:, off:512].unsqueeze(1).broadcast_to(
                        [128, 2, 512 - off]))
            return pX

        def emit_attn_ib(pair, ib, fillers=(), pre_px=None, post_fillers=(),
                         chain_copy_on_act=False):
            n_jb = 4 * (ib + 1)
            oA = acc.tile([65, 512], f32, tag="acc", name=f"oA{pair}{ib}")
            oB = acc.tile([65, 512], f32, tag="acc", name=f"oB{pair}{ib}")
            fl = list(fillers)
            nf = len(fl)
            pos = [i * n_jb // nf for i in range(nf)] if nf else []
            fi = 0
            for jb in range(n_jb):
                while fi < nf and pos[fi] == jb:
                    fl[fi]()
                    fi += 1
                off = max(0, 128 * (jb - 4 * ib))
                pX = (pre_px.get(jb) if pre_px is not None else None)
                if pX is None:
                    pX = emit_qkexp(pair, ib, jb)
                for h01, oX in ((0, oA), (1, oB)):
                    nc.tensor.matmul(
                        oX[:, off:512],
                        lhsT=vaug[jb][:, 2 * pair + h01, :],
                        rhs=pX[:, 512 * h01 + off:512 * (h01 + 1)],
                        start=(jb == 0), stop=(jb == n_jb - 1))
            while fi < nf:
                fl[fi]()
                fi += 1
            # softmax-normalization chain: copy the PSUM sums row to SBUF
            # (custom-DVE recip reads garbage from PSUM), reciprocal,
            # broadcast on gpsimd, multiply into attnT on DVE. For the very
            # last slot the copies run on the (by then idle) scalar engine
            # instead of the mask-mul-backlogged DVE.
            rss = []
            for h01, oX in ((0, oA), (1, oB)):
                tmp = rsp.tile([1, 512], f32, tag="rtmp",
                               name=f"rt{pair}{ib}{h01}")
                if chain_copy_on_act:
                    nc.scalar.copy(tmp[:], oX[64:65, :])
                else:
                    nc.vector.tensor_copy(tmp[:], oX[64:65, :])
                rs = rsp.tile([1, 512], f32, tag="rsp",
                              name=f"rs{pair}{ib}{h01}")
                nc.vector.reciprocal_approx_fast(rs[:], tmp[:])
                rss.append(rs)
            bcss = []
            for h01 in (0, 1):
                bcs = bcsp.tile([64, 512], f32, tag="bcsp",
                                name=f"bcs{pair}{ib}{h01}")
                nc.gpsimd.partition_broadcast(bcs[:], rss[h01][:])
                bcss.append(bcs)
            for h01, oX in ((0, oA), (1, oB)):
                nc.vector.tensor_mul(
                    attnT[pair][64 * h01:64 * (h01 + 1),
                                512 * ib:512 * (ib + 1)],
                    oX[0:64, :], bcss[h01][:])
            for f in post_fillers:
                f()

        # ---- emission schedule ----------------------------------------
        warm_ctr = [0]

        def warm(n):
            # Dummy matmuls on the zeroed scratch tile: PE clock warm-up /
            # keep-alive during windows with no runnable real work.
            for _ in range((n + 1) // 2):
                w = warm_ctr[0]
                warm_ctr[0] += 1
                wps = mm.tile([128, 1024], f32, tag="mm", name=f"warmps{w}")
                nc.tensor.matmul(wps[:, 0:512], lhsT=scratch[:, 0:128],
                                 rhs=scratch[:], start=True, stop=True)
                nc.tensor.matmul(wps[:, 512:1024], lhsT=scratch[:, 0:128],
                                 rhs=scratch[:], start=True, stop=True)

        # PE clock warm-up: runnable as soon as the program starts (no DMA
        # dependency); sized to end right as the first input DMAs land.
        warm(26)

        # Startup: qk(0)-sc0 as soon as its DMAs land, then ib0/ib1
        # scores+exp interleaved with the remaining sc1/v units so the PE
        # never drains while ACT works through the first exps.
        qk_unit(0, 0, 0)()
        qk_unit(0, 1, 0)()
        pre0 = {0: emit_qkexp(0, 0, 0), 1: emit_qkexp(0, 0, 1)}
        qk_unit(0, 0, 1)()
        pre0[2] = emit_qkexp(0, 0, 2)
        pre0[3] = emit_qkexp(0, 0, 3)
        qk_unit(0, 1, 1)()
        pre1 = {0: emit_qkexp(0, 1, 0), 1: emit_qkexp(0, 1, 1)}
        v_unit(0)()
        pre1[2] = emit_qkexp(0, 1, 2)
        pre1[3] = emit_qkexp(0, 1, 3)
        v_unit(1)()
        v_unit(2)()
        v_unit(3)()

        # pair 0 pair-major (its fillers only need xt/wvt, which land early)
        emit_attn_ib(0, 0, pre_px=pre0)
        emit_attn_ib(0, 1, [v_unit(4), v_unit(5), v_unit(6), v_unit(7),
                            qk_unit(0, 0, 2), qk_unit(0, 1, 2)],
                     pre_px=pre1)
        emit_attn_ib(0, 2, [v_unit(8), v_unit(9), v_unit(10), v_unit(11),
                            qk_unit(0, 0, 3), qk_unit(0, 1, 3)])
        emit_attn_ib(0, 3, [qk_unit(1, 0, 0), qk_unit(1, 1, 0), v_unit(12),
                            qk_unit(1, 0, 1), v_unit(13),
                            qk_unit(1, 1, 1), v_unit(14), v_unit(15)])

        # pairs 1-3 round-major (ib-major): spreads the out-proj filler
        # across the whole back half so the PE never starves and the clock
        # stays at the full p-state.
        emit_attn_ib(1, 0, [qk_unit(2, 0, 0), qk_unit(2, 1, 0)])
        emit_attn_ib(2, 0, [qk_unit(3, 0, 0), qk_unit(3, 1, 0)])
        emit_attn_ib(3, 0, [qk_unit(1, 0, 2), qk_unit(1, 1, 2)])

        emit_attn_ib(1, 1, [qk_unit(2, 0, 1), qk_unit(2, 1, 1),
                            op_unit(0, 0), op_unit(0, 1)])
        emit_attn_ib(2, 1, [qk_unit(3, 0, 1), qk_unit(3, 1, 1),
                            op_unit(1, 0)])
        emit_attn_ib(3, 1, [qk_unit(1, 0, 3), qk_unit(1, 1, 3),
                            op_unit(1, 1)])

        emit_attn_ib(1, 2, [qk_unit(2, 0, 2), qk_unit(2, 1, 2),
                            op_unit(2, 0), op_unit(2, 1), op_unit(3, 0)])
        emit_attn_ib(2, 2, [qk_unit(3, 0, 2), qk_unit(3, 1, 2),
                            op_unit(3, 1), op_unit(4, 0)])
        emit_attn_ib(3, 2, [op_unit(4, 1), op_unit(5, 0), op_unit(5, 1)])

        emit_attn_ib(1, 3, [qk_unit(2, 0, 3), qk_unit(2, 1, 3),
                            op_unit(6, 0), op_unit(6, 1), op_unit(7, 0),
                            op_unit(7, 1)])
        emit_attn_ib(2, 3, [qk_unit(3, 0, 3), qk_unit(3, 1, 3),
                            op_unit(8, 0), op_unit(8, 1), op_unit(9, 0)])
        emit_attn_ib(3, 3, [op_unit(9, 1), op_unit(10, 0), op_unit(10, 1)],
                     post_fillers=[op_unit(11, 0), op_unit(11, 1)],
                     chain_copy_on_act=True)
        # keep the PE clock hot while the final normalization chain
        # (DVE/gpsimd latency) blocks the tail out-proj units
        warm(20)
        for i, s in enumerate((12, 13, 14, 15)):
            op_unit(s, 0, alt_pool=(i % 2 == 1), split_dma=True)()
            op_unit(s, 1, alt_pool=(i % 2 == 1), split_dma=True)()

    nc.compile()
    return nc


def _get_nc():
    if "nc" not in _CACHE:
        _CACHE["nc"] = _build()
    return _CACHE["nc"]


def _shard_inputs(x, w_qkv, w_out):
    bf = ml_dtypes.bfloat16
    in_maps = []
    for c in range(N_CORES):
        b, g = divmod(c, 2)
        xT = np.ascontiguousarray(x[b].T).astype(bf)
        wq = w_qkv[512 * g:512 * (g + 1)]
        wk = w_qkv[1024 + 512 * g:1024 + 512 * (g + 1)]
        wv = w_qkv[2048 + 512 * g:2048 + 512 * (g + 1)]
        blocks = [wq[0:128], wk[0:128], wv]
        for p in range(1, 4):
            blocks.append(wq[128 * p:128 * (p + 1)])
            blocks.append(wk[128 * p:128 * (p + 1)])
        wqkvT = np.ascontiguousarray(
            np.concatenate(blocks, axis=0).T).astype(bf)
        woutT = np.ascontiguousarray(w_out[:, 512 * g:512 * (g + 1)].T
                                     ).astype(bf)
        in_maps.append({"xT": xT, "wqkvT": wqkvT, "woutT": woutT})
    return in_maps


def kernel(x, w_qkv, w_out):
    global LAST_EXEC_TIME_NS
    from concourse.bass_utils import run_bass_kernel_spmd

    nc = _get_nc()
    in_maps = _shard_inputs(np.asarray(x, dtype=np.float32),
                            np.asarray(w_qkv, dtype=np.float32),
                            np.asarray(w_out, dtype=np.float32))
    trace = bool(int(os.environ.get("KBENCH_TRACE", "0")))
    res = run_bass_kernel_spmd(nc, in_maps, list(range(N_CORES)), trace=trace)
    LAST_EXEC_TIME_NS = res.exec_time_ns
    out = np.empty((4, S, D), dtype=np.float32)
    for b in range(4):
        out[b] = res.results[2 * b]["out"] + res.results[2 * b + 1]["out"]
    return out


# revision 21
# speedup vs baseline: 1.0047x; 1.0047x over previous
"""Causal self-attention (B=4, S=2048, D=1024, H=16, hd=64) on 8 TRN2 cores.

Sharding: core c = (batch b = c//2, head-group g = c%2); each core computes
8 heads for one batch. Out-projection partials are summed on host (the only
cross-shard reduction).

Device kernel layout (all matmul contractions have the contracted dim on
SBUF partitions; everything stays transposed so no on-device transposes):
  qT,kT  [64*2heads, S]  = wqkvT-chunk.T @ xT          (stationary weights)
  v_aug  [S-block, 8*65] = xT-chunk.T @ wvT (+ ones col per head for sums)
  sT     [j 128, i 512]  = kT-slice.T @ qT-slice        (2 heads row-packed)
  pT     = exp(sT/8)  bf16 via ACT; diagonal blocks masked by DVE mask-mul
  outT   [65, i]        += v_aug.T @ pT   (row 64 accumulates softmax sums)
  attnT  = outT * bcast(1/sums)           (gpsimd partition_broadcast)
  out    [s 128, e]      = attnT-chunk.T @ woutT-chunk  (accum over c-chunks)

Scheduling: the exp on the scalar engine is the rate limiter of the
attention inner loop (~1.07us per 128-k-row block vs ~0.75us of PE work),
so all non-attention PE work (QKV projection groups, V blocks, out-proj
groups) is emitted as FILLER UNITS spread inside the attention jb-loops;
the PE then never idles and its clock stays at the full p-state. Input
DMAs are issued in dependency-priority order (pair-0 qk weights + first
half of xT first) so the first exp lands ~20us into the kernel. All
PSUM->SBUF copies run on DVE, keeping the scalar engine exp-only.
"""
import sys
import os

sys.path.insert(0, "/opt/trn_rl_repo")

import numpy as np
import ml_dtypes
from contextlib import ExitStack

S = 2048
D = 1024
HL = 8          # heads per core
HD = 64
PAIRS = 4       # head pairs per core
NIB = 4         # i-blocks of 512
N_CORES = 8

_CACHE = {}
LAST_EXEC_TIME_NS = None


def _build():
    import concourse.tile as tile
    import concourse.mybir as mybir
    from concourse import bacc

    bf = mybir.dt.bfloat16
    f32 = mybir.dt.float32
    EXP = mybir.ActivationFunctionType.Exp
    GE = mybir.AluOpType.is_ge

    nc = bacc.Bacc("TRN2", target_bir_lowering=False, debug=False,
                   num_devices=N_CORES)
    xT_d = nc.dram_tensor("xT", [D, S], bf, kind="ExternalInput").ap()
    # host-side column layout: [q0|k0 (256) | v (512) | q1 k1 q2 k2 q3 k3]
    wqkvT_d = nc.dram_tensor("wqkvT", [D, 3 * 512], bf,
                             kind="ExternalInput").ap()
    woutT_d = nc.dram_tensor("woutT", [512, D], bf, kind="ExternalInput").ap()
    out_d = nc.dram_tensor("out", [S, D], f32, kind="ExternalOutput").ap()

    xv = xT_d.rearrange("(c p) s -> p c s", p=128)          # [128, 8, 2048]
    wv_ = wqkvT_d.rearrange("(c p) n -> p c n", p=128)      # [128, 8, 1536]
    wo_ = woutT_d.rearrange("(c p) e -> p c e", p=128)      # [128, 4, 1024]

    with tile.TileContext(nc) as tc, ExitStack() as ctx:
        sb = ctx.enter_context(tc.tile_pool(name="sb", bufs=1))
        # PSUM budget (8 banks): mm 2x[128,1024] (4) + acc 2x[65,512] (2)
        # + unit 2x[128,512] (2).
        mm = ctx.enter_context(tc.tile_pool(name="mm", bufs=2, space="PSUM"))
        acc = ctx.enter_context(tc.tile_pool(name="acc", bufs=2,
                                             space="PSUM"))
        unit = ctx.enter_context(tc.tile_pool(name="unit", bufs=2,
                                              space="PSUM"))
        pp = ctx.enter_context(tc.tile_pool(name="pp", bufs=8))
        rsp = ctx.enter_context(tc.tile_pool(name="rsp", bufs=4))
        bcsp = ctx.enter_context(tc.tile_pool(name="bcsp", bufs=4))
        osbp = ctx.enter_context(tc.tile_pool(name="osbp", bufs=4))

        # ---- persistent SBUF tiles -------------------------------------
        xt0 = sb.tile([128, 8, 512], bf, tag="xt0", name="xt0")
        xt1 = sb.tile([128, 8, 512], bf, tag="xt1", name="xt1")
        xt23 = sb.tile([128, 8, 1024], bf, tag="xt23", name="xt23")
        wqk0 = sb.tile([128, 8, 2, 128], bf, tag="wqk0", name="wqk0")
        wvt = sb.tile([128, 8, 512], bf, tag="wvt", name="wvt")
        wqkR = sb.tile([128, 8, 6, 128], bf, tag="wqkR", name="wqkR")
        wout = sb.tile([128, 4, 1024], bf, tag="wout", name="wout")
        qT = [sb.tile([128, S], bf, tag=f"qT{p}", name=f"qT{p}")
              for p in range(PAIRS)]
        kT = [sb.tile([128, S], bf, tag=f"kT{p}", name=f"kT{p}")
              for p in range(PAIRS)]
        vaug = [sb.tile([128, HL, HD + 1], bf, tag=f"vaug{s}",
                        name=f"vaug{s}") for s in range(16)]
        attnT = [sb.tile([128, S], bf, tag=f"attnT{p}", name=f"attnT{p}")
                 for p in range(PAIRS)]
        masks = [sb.tile([128, 512], bf, tag=f"mask{m}", name=f"mask{m}")
                 for m in range(4)]

        scratch = sb.tile([128, 512], bf, tag="warm", name="warm")

        # ---- init (gpsimd; independent of DMAs). Order: warm-up scratch
        # first (unblocks PE clock warm-up), masks next (needed by the
        # first diagonal-block mask-mul ~17us in), vaug last.
        nc.gpsimd.memset(scratch[:], 0.0)
        for m in range(4):
            nc.gpsimd.memset(masks[m][:], 1.0)
            nc.gpsimd.affine_select(
                out=masks[m][:], in_=masks[m][:], compare_op=GE, fill=0.0,
                base=-128 * m, channel_multiplier=-1, pattern=[[1, 512]])
        for s in range(16):
            nc.gpsimd.memset(vaug[s][:], 1.0)

        # ---- input DMAs in dependency-priority order -------------------
        nc.sync.dma_start(
            wqk0[:], wv_[:, :, 0:256].rearrange("p c (w n) -> p c w n",
                                                n=128))
        nc.sync.dma_start(xt0[:], xv[:, :, 0:512])
        nc.sync.dma_start(wvt[:], wv_[:, :, 256:768])
        nc.sync.dma_start(xt1[:], xv[:, :, 512:1024])
        nc.sync.dma_start(xt23[:], xv[:, :, 1024:2048])
        nc.sync.dma_start(
            wqkR[:], wv_[:, :, 768:1536].rearrange("p c (w n) -> p c w n",
                                                   n=128))
        nc.sync.dma_start(wout[:], wo_[:])

        def xt_rhs(dc, sc):
            if sc == 0:
                return xt0[:, dc, :]
            if sc == 1:
                return xt1[:, dc, :]
            return xt23[:, dc, 512 * (sc % 2):512 * (sc % 2 + 1)]

        def xt_vlhs(dc, sblk):
            sc = sblk // 4
            r = 128 * (sblk % 4)
            if sc == 0:
                return xt0[:, dc, r:r + 128]
            if sc == 1:
                return xt1[:, dc, r:r + 128]
            base = 512 * (sc % 2) + r
            return xt23[:, dc, base:base + 128]

        def wqk_lhs(pair, w, dc):
            if pair == 0:
                return wqk0[:, dc, w, :]
            return wqkR[:, dc, 2 * (pair - 1) + w, :]

        # ---- filler units (each = one PSUM group + a DVE drain) --------
        def v_unit(sblk):
            def emit():
                ps = unit.tile([128, 512], f32, tag="u", name=f"vps{sblk}")
                for dc in range(8):
                    nc.tensor.matmul(ps[:], lhsT=xt_vlhs(dc, sblk),
                                     rhs=wvt[:, dc, :],
                                     start=(dc == 0), stop=(dc == 7))
                nc.vector.tensor_copy(
                    vaug[sblk][:, :, 0:64],
                    ps[:].rearrange("p (h d) -> p h d", h=HL))
            return emit

        def qk_unit(pair, w, sc):
            def emit():
                dest = qT[pair] if w == 0 else kT[pair]
                ps = unit.tile([128, 512], f32, tag="u",
                               name=f"qkps{pair}{w}{sc}")
                for dc in range(8):
                    nc.tensor.matmul(ps[:], lhsT=wqk_lhs(pair, w, dc),
                                     rhs=xt_rhs(dc, sc),
                                     start=(dc == 0), stop=(dc == 7))
                nc.vector.tensor_copy(dest[:, 512 * sc:512 * (sc + 1)],
                                      ps[:])
            return emit

        osb_tiles = {}

        def op_unit(sblk, eh, alt_pool=False, split_dma=False):
            def emit():
                if eh == 0:
                    osb_tiles[sblk] = osbp.tile([128, D], f32, tag="osb",
                                                name=f"osb{sblk}")
                osb = osb_tiles[sblk]
                if alt_pool:
                    pst = mm.tile([128, 1024], f32, tag="mm",
                                  name=f"ops{sblk}{eh}")
                    ps_ap = pst[:, 0:512]
                else:
                    pst = unit.tile([128, 512], f32, tag="u",
                                    name=f"ops{sblk}{eh}")
                    ps_ap = pst[:]
                for cc in range(4):
                    nc.tensor.matmul(
                        ps_ap,
                        lhsT=attnT[cc][:, 128 * sblk:128 * (sblk + 1)],
                        rhs=wout[:, cc, 512 * eh:512 * (eh + 1)],
                        start=(cc == 0), stop=(cc == 3))
                nc.vector.tensor_copy(osb[:, 512 * eh:512 * (eh + 1)],
                                      ps_ap)
                if split_dma:
                    nc.sync.dma_start(
                        out_d[128 * sblk:128 * (sblk + 1),
                              512 * eh:512 * (eh + 1)],
                        osb[:, 512 * eh:512 * (eh + 1)])
                elif eh == 1:
                    nc.sync.dma_start(
                        out_d[128 * sblk:128 * (sblk + 1), :], osb[:])
            return emit

        # ---- attention -------------------------------------------------
        def emit_qkexp(pair, ib, jb):
            off = max(0, 128 * (jb - 4 * ib))
            s2 = mm.tile([128, 1024], f32, tag="mm",
                         name=f"s2_{pair}{ib}{jb}")
            for h01 in range(2):
                r0, r1 = 64 * h01, 64 * (h01 + 1)
                nc.tensor.matmul(
                    s2[:, 512 * h01 + off:512 * (h01 + 1)],
                    lhsT=kT[pair][r0:r1, 128 * jb:128 * (jb + 1)],
                    rhs=qT[pair][r0:r1, 512 * ib + off:512 * (ib + 1)],
                    start=True, stop=True)
            pX = pp.tile([128, 1024], bf, tag="pp", name=f"pX{pair}{ib}{jb}")
            s3 = s2[:].rearrange("p (h i) -> p h i", h=2)
            p3 = pX[:].rearrange("p (h i) -> p h i", h=2)
            nc.scalar.activation(p3[:, :, off:512], s3[:, :, off:512],
                                 EXP, scale=0.125)
            if jb >= 4 * ib:
                m = jb - 4 * ib
                nc.vector.tensor_mul(
                    p3[:, :, off:512], p3[:, :, off:512],
                    masks[m][:, off:512].unsqueeze(1).broadcast_to(
                        [128, 2, 512 - off]))
            return pX

        def emit_attn_ib(pair, ib, fillers=(), pre_px=None, post_fillers=(),
                         chain_copy_on_act=False):
            n_jb = 4 * (ib + 1)
            oA = acc.tile([65, 512], f32, tag="acc", name=f"oA{pair}{ib}")
            oB = acc.tile([65, 512], f32, tag="acc", name=f"oB{pair}{ib}")
            fl = list(fillers)
            nf = len(fl)
            pos = [i * n_jb // nf for i in range(nf)] if nf else []
            fi = 0
            for jb in range(n_jb):
                while fi < nf and pos[fi] == jb:
                    fl[fi]()
                    fi += 1
                off = max(0, 128 * (jb - 4 * ib))
                pX = (pre_px.get(jb) if pre_px is not None else None)
                if pX is None:
                    pX = emit_qkexp(pair, ib, jb)
                for h01, oX in ((0, oA), (1, oB)):
                    nc.tensor.matmul(
                        oX[:, off:512],
                        lhsT=vaug[jb][:, 2 * pair + h01, :],
                        rhs=pX[:, 512 * h01 + off:512 * (h01 + 1)],
                        start=(jb == 0), stop=(jb == n_jb - 1))
            while fi < nf:
                fl[fi]()
                fi += 1
            # softmax-normalization chain: copy the PSUM sums row to SBUF
            # (custom-DVE recip reads garbage from PSUM), reciprocal,
            # broadcast on gpsimd, multiply into attnT on DVE. For the very
            # last slot the copies run on the (by then idle) scalar engine
            # instead of the mask-mul-backlogged DVE.
            rss = []
            for h01, oX in ((0, oA), (1, oB)):
                tmp = rsp.tile([1, 512], f32, tag="rtmp",
                               name=f"rt{pair}{ib}{h01}")
                if chain_copy_on_act:
                    nc.scalar.copy(tmp[:], oX[64:65, :])
                else:
                    nc.vector.tensor_copy(tmp[:], oX[64:65, :])
                rs = rsp.tile([1, 512], f32, tag="rsp",
                              name=f"rs{pair}{ib}{h01}")
                nc.vector.reciprocal_approx_fast(rs[:], tmp[:])
                rss.append(rs)
            bcss = []
            for h01 in (0, 1):
                bcs = bcsp.tile([64, 512], f32, tag="bcsp",
                                name=f"bcs{pair}{ib}{h01}")
                nc.gpsimd.partition_broadcast(bcs[:], rss[h01][:])
                bcss.append(bcs)
            for h01, oX in ((0, oA), (1, oB)):
                nc.vector.tensor_mul(
                    attnT[pair][64 * h01:64 * (h01 + 1),
                                512 * ib:512 * (ib + 1)],
                    oX[0:64, :], bcss[h01][:])
            for f in post_fillers:
                f()

        # ---- emission schedule ----------------------------------------
        warm_ctr = [0]

        def warm(n):
            # Dummy matmuls on the zeroed scratch tile: PE clock warm-up /
            # keep-alive during windows with no runnable real work.
            for _ in range((n + 1) // 2):
                w = warm_ctr[0]
                warm_ctr[0] += 1
                wps = mm.tile([128, 1024], f32, tag="mm", name=f"warmps{w}")
                nc.tensor.matmul(wps[:, 0:512], lhsT=scratch[:, 0:128],
                                 rhs=scratch[:], start=True, stop=True)
                nc.tensor.matmul(wps[:, 512:1024], lhsT=scratch[:, 0:128],
                                 rhs=scratch[:], start=True, stop=True)

        # PE clock warm-up: runnable as soon as the program starts (no DMA
        # dependency); sized to end right as the first input DMAs land.
        warm(26)

        # Startup: qk(0)-sc0 as soon as its DMAs land, then ib0/ib1
        # scores+exp interleaved with the remaining sc1/v units so the PE
        # never drains while ACT works through the first exps.
        qk_unit(0, 0, 0)()
        qk_unit(0, 1, 0)()
        pre0 = {0: emit_qkexp(0, 0, 0), 1: emit_qkexp(0, 0, 1)}
        qk_unit(0, 0, 1)()
        pre0[2] = emit_qkexp(0, 0, 2)
        pre0[3] = emit_qkexp(0, 0, 3)
        qk_unit(0, 1, 1)()
        pre1 = {0: emit_qkexp(0, 1, 0), 1: emit_qkexp(0, 1, 1)}
        warm(6)
        v_unit(0)()
        pre1[2] = emit_qkexp(0, 1, 2)
        pre1[3] = emit_qkexp(0, 1, 3)
        v_unit(1)()
        v_unit(2)()
        v_unit(3)()

        # pair 0 pair-major (its fillers only need xt/wvt, which land early)
        emit_attn_ib(0, 0, pre_px=pre0)
        emit_attn_ib(0, 1, [v_unit(4), v_unit(5), v_unit(6), v_unit(7),
                            qk_unit(0, 0, 2), qk_unit(0, 1, 2)],
                     pre_px=pre1)
        emit_attn_ib(0, 2, [v_unit(8), qk_unit(0, 0, 3), v_unit(9),
                            qk_unit(1, 0, 0), v_unit(10), qk_unit(1, 1, 0),
                            v_unit(11), qk_unit(0, 1, 3)])
        emit_attn_ib(0, 3, [qk_unit(1, 0, 1), v_unit(12),
                            qk_unit(1, 1, 1), v_unit(13), v_unit(14),
                            v_unit(15)])

        # pairs 1-3 round-major (ib-major): spreads the out-proj filler
        # across the whole back half so the PE never starves and the clock
        # stays at the full p-state.
        emit_attn_ib(1, 0, [qk_unit(2, 0, 0), qk_unit(2, 1, 0)])
        emit_attn_ib(2, 0, [qk_unit(3, 0, 0), qk_unit(3, 1, 0)])
        emit_attn_ib(3, 0, [qk_unit(1, 0, 2), qk_unit(1, 1, 2)])

        emit_attn_ib(1, 1, [qk_unit(2, 0, 1), qk_unit(2, 1, 1),
                            op_unit(0, 0), op_unit(0, 1)])
        emit_attn_ib(2, 1, [qk_unit(3, 0, 1), qk_unit(3, 1, 1),
                            op_unit(1, 0)])
        emit_attn_ib(3, 1, [qk_unit(1, 0, 3), qk_unit(1, 1, 3),
                            op_unit(1, 1)])

        emit_attn_ib(1, 2, [qk_unit(2, 0, 2), qk_unit(2, 1, 2),
                            op_unit(2, 0), op_unit(2, 1), op_unit(3, 0)])
        emit_attn_ib(2, 2, [qk_unit(3, 0, 2), qk_unit(3, 1, 2),
                            op_unit(3, 1), op_unit(4, 0)])
        emit_attn_ib(3, 2, [op_unit(4, 1), op_unit(5, 0), op_unit(5, 1)])

        emit_attn_ib(1, 3, [qk_unit(2, 0, 3), qk_unit(2, 1, 3),
                            op_unit(6, 0), op_unit(6, 1), op_unit(7, 0),
                            op_unit(7, 1)])
        emit_attn_ib(2, 3, [qk_unit(3, 0, 3), qk_unit(3, 1, 3),
                            op_unit(8, 0), op_unit(8, 1), op_unit(9, 0)])
        emit_attn_ib(3, 3, [op_unit(9, 1), op_unit(10, 0), op_unit(10, 1)],
                     post_fillers=[op_unit(11, 0), op_unit(11, 1)],
                     chain_copy_on_act=True)
        # keep the PE clock hot while the final normalization chain
        # (DVE/gpsimd latency) blocks the tail out-proj units
        warm(20)
        for i, s in enumerate((12, 13, 14, 15)):
            op_unit(s, 0, alt_pool=(i % 2 == 1), split_dma=True)()
            op_unit(s, 1, alt_pool=(i % 2 == 1), split_dma=True)()

    nc.compile()
    return nc


def _get_nc():
    if "nc" not in _CACHE:
        _CACHE["nc"] = _build()
    return _CACHE["nc"]


def _shard_inputs(x, w_qkv, w_out):
    bf = ml_dtypes.bfloat16
    in_maps = []
    for c in range(N_CORES):
        b, g = divmod(c, 2)
        xT = np.ascontiguousarray(x[b].T).astype(bf)
        wq = w_qkv[512 * g:512 * (g + 1)]
        wk = w_qkv[1024 + 512 * g:1024 + 512 * (g + 1)]
        wv = w_qkv[2048 + 512 * g:2048 + 512 * (g + 1)]
        blocks = [wq[0:128], wk[0:128], wv]
        for p in range(1, 4):
            blocks.append(wq[128 * p:128 * (p + 1)])
            blocks.append(wk[128 * p:128 * (p + 1)])
        wqkvT = np.ascontiguousarray(
            np.concatenate(blocks, axis=0).T).astype(bf)
        woutT = np.ascontiguousarray(w_out[:, 512 * g:512 * (g + 1)].T
                                     ).astype(bf)
        in_maps.append({"xT": xT, "wqkvT": wqkvT, "woutT": woutT})
    return in_maps


def kernel(x, w_qkv, w_out):
    global LAST_EXEC_TIME_NS
    from concourse.bass_utils import run_bass_kernel_spmd

    nc = _get_nc()
    in_maps = _shard_inputs(np.asarray(x, dtype=np.float32),
                            np.asarray(w_qkv, dtype=np.float32),
                            np.asarray(w_out, dtype=np.float32))
    trace = bool(int(os.environ.get("KBENCH_TRACE", "0")))
    res = run_bass_kernel_spmd(nc, in_maps, list(range(N_CORES)), trace=trace)
    LAST_EXEC_TIME_NS = res.exec_time_ns
    out = np.empty((4, S, D), dtype=np.float32)
    for b in range(4):
        out[b] = res.results[2 * b]["out"] + res.results[2 * b + 1]["out"]
    return out
